# revision 1
# baseline (speedup 1.0000x reference)
"""Trainium2 Bass kernel for nn_MemTransformerLM (Transformer-XL layer).

Sharding (8 cores): batch (4) x head-half (2). Every core runs an identical
program over batch b = c//2 and heads [hh*8, hh*8+8), hh = c%2, for all 1024
queries. After o_proj a 2-rank ReduceScatter over core pairs (2b, 2b+1)
splits tokens for the FFN: even core keeps tokens [0,512), odd [512,1024).

Attention rel-shift: BD[i,j] = BD_raw[i, j-i+Q-1] is applied with a
"diagonal" SBUF->SBUF DMA (flat access pattern [[W-1,128],[1,N]]) that
accumulates the shifted BD window into the AC scores. The causal mask is
baked in by memsetting the out-of-range tail of each BD window to -30
before the shift, so exp() zeroes masked lanes without a mask pass.
"""

import contextlib
import math

import numpy as np

import concourse.bass as bass
import concourse.bacc as bacc
import concourse.mybir as mybir
import concourse.tile as tile
from concourse.masks import make_identity

F32 = mybir.dt.float32
BF16 = mybir.dt.bfloat16
AF = mybir.ActivationFunctionType
ALU = mybir.AluOpType


class Cfg:
    D = 1024      # model dim
    NHC = 8       # heads per core
    DH = 64       # head dim
    KL = 2048     # key length
    Q = 1024      # query length
    DI = 4096     # ffn inner
    LN_EPS = 1e-5
    N_CORES = 8

    HD = property(lambda s: s.NHC * s.DH)       # head dims per core
    SCALE = property(lambda s: 1.0 / (s.DH ** 0.5))
    M = property(lambda s: s.KL - s.Q)          # mem length
    NS = property(lambda s: s.Q // 128)         # q tiles
    NJT = property(lambda s: s.KL // 128)       # key tiles
    DPT = property(lambda s: s.D // 128)
    HPT = property(lambda s: s.HD // 128)
    NTT = property(lambda s: s.KL // 128)
    WB = property(lambda s: s.KL + 128)         # BD window buffer width
    TOKF = property(lambda s: s.Q // 2)         # ffn tokens per core

    def jmax(self, s):
        return min(self.KL, 128 * (s + 1) + self.M)

    def jcomp(self, s):
        return min(self.KL, -(-self.jmax(s) // 512) * 512)

    def wstart(self, s):
        return self.Q - 128 * (s + 1)

    def wreal(self, s):
        return min(self.jcomp(s) + 128, self.KL - self.wstart(s))


def ts(i, n):
    return slice(i * n, (i + 1) * n)


def chunks(total, sz=512):
    return [(lo, min(total, lo + sz)) for lo in range(0, total, sz)]


def build_kernel(c: Cfg = None, collective=True):
    c = c or Cfg()
    nc = bacc.Bacc("TRN2", target_bir_lowering=False)

    io = {}
    def din(name, shape):
        io[name] = nc.dram_tensor(name, shape, F32, kind="ExternalInput")
    din("xw", [c.KL, c.D])
    din("r_in", [c.KL, c.D])
    din("qkvw", [c.D, 3 * c.HD])
    din("rnetw", [c.D, c.HD])
    din("oww", [c.HD, c.D])
    din("rwb", [1, c.HD])
    din("rrb", [1, c.HD])
    din("ln1g", [1, c.D]); din("ln1b", [1, c.D])
    din("ln2g", [1, c.D]); din("ln2b", [1, c.D])
    din("ffw1", [c.D, c.DI]); din("ffb1", [1, c.DI])
    din("ffw2", [c.DI, c.D]); din("ffb2", [1, c.D])
    din("wres", [c.TOKF, c.D])
    io["out"] = nc.dram_tensor("out", [c.TOKF, c.D], F32, kind="ExternalOutput")
    io["rs_bin"] = nc.dram_tensor("rs_bin", [c.Q, c.D], F32)
    io["rs_bout"] = nc.dram_tensor("rs_bout", [c.TOKF, c.D], F32)

    with tile.TileContext(nc) as tc:
        _body(tc, nc, c, io, collective=collective)
    nc.finalize()
    return nc


def _qslice(buf, c, hp, hr, s):
    """[64,128] lhsT slice for head (hp, hr) and q-tile s of a [128, HPT*Q] buf."""
    return buf[hr:hr + 64, hp * c.Q + s * 128: hp * c.Q + (s + 1) * 128]


def _body(tc, nc, c, io, collective=True):
    ctx = contextlib.ExitStack()
    rg = [[i, i + 1] for i in range(0, c.N_CORES, 2)]
    with ctx:
        small = ctx.enter_context(tc.tile_pool(name="small", bufs=4))
        psA = ctx.enter_context(tc.tile_pool(name="psA", bufs=2, space="PSUM"))
        psB = ctx.enter_context(tc.tile_pool(name="psB", bufs=2, space="PSUM"))

        def ps_a():
            return psA.tile([128, 1024], F32, tag="a", name="psa")

        def ps_b():
            return psB.tile([128, 1024], F32, tag="b", name="psb")

        keep = ctx.enter_context(tc.tile_pool(name="keep", bufs=1))
        ident = keep.tile([128, 128], BF16, tag="identb")
        make_identity(nc, ident)
        identf = keep.tile([128, 128], F32, tag="identf")
        make_identity(nc, identf)

        def load_transposed(src, dst, stage):
            """src DRAM [KL, D] fp32 -> dst [128, DPT*KL] bf16 transposed."""
            for tt in range(c.NTT):
                f32t = stage.tile([128, c.D], F32, tag="ldA")
                nc.sync.dma_start(out=f32t[:], in_=src[ts(tt, 128), :])
                bft = stage.tile([128, c.D], BF16, tag="castA")
                nc.vector.tensor_copy(out=bft[:], in_=f32t[:])
                dstap = bass.AP(
                    tensor=dst.tensor, offset=dst.offset + tt * 128,
                    ap=[[c.DPT * c.KL, 128], [c.KL, c.DPT], [1, 128]])
                nc.sync.dma_start(out=dstap, in_=bft[:], transpose=True)

        def stream_w(src, ptiles, width, tag, stage, wpool):
            """load fp32 weight rows by 128-ptile, cast to bf16 tiles (kept)."""
            tiles = []
            for p in range(ptiles):
                f32t = stage.tile([128, width], F32, tag="ldW_" + tag)
                nc.sync.dma_start(out=f32t[:], in_=src[ts(p, 128), :])
                bt = wpool.tile([128, width], BF16, tag="%s_%d" % (tag, p))
                nc.vector.tensor_copy(out=bt[:], in_=f32t[:])
                tiles.append(bt)
            return tiles

        # ============ phase A/B: R^T first (rT freed before xT) ============
        atp = tc.alloc_tile_pool(name="atp", bufs=1)
        attk = tc.alloc_tile_pool(name="attk", bufs=1)
        rTp = attk.tile([128, c.HPT * c.WB], BF16, tag="rTp")
        kT = attk.tile([128, c.HPT * c.KL], BF16, tag="kT")
        VW = c.NHC * 65
        vb = attk.tile([128, c.NTT * VW], BF16, tag="vb")
        rwq = attk.tile([128, c.HPT * c.Q], BF16, tag="rwq")
        rrq = attk.tile([128, c.HPT * c.Q], BF16, tag="rrq")
        attnT = atp.tile([128, c.HPT * c.Q], BF16, tag="attnT")

        with tc.tile_pool(name="phR", bufs=1) as phR, \
             tc.tile_pool(name="stageR", bufs=3) as stage:
            rT = phR.tile([128, c.DPT * c.KL], BF16, tag="rT")
            load_transposed(io["r_in"], rT, stage)
            with tc.tile_pool(name="wpoolR", bufs=1) as wpool:
                wr_t = stream_w(io["rnetw"], c.DPT, c.HD, "wr", stage, wpool)
                for m in range(c.HPT):
                    nc.gpsimd.memset(rTp[:, m * c.WB + c.KL:(m + 1) * c.WB], 0.0)
                    for lo, hi in chunks(c.KL):
                        ps = ps_a()
                        for k in range(c.DPT):
                            nc.tensor.matmul(
                                ps[:, 0:hi - lo], wr_t[k][:, ts(m, 128)],
                                rT[:, k * c.KL + lo: k * c.KL + hi],
                                start=(k == 0), stop=(k == c.DPT - 1))
                        nc.scalar.activation(
                            out=rTp[:, m * c.WB + lo: m * c.WB + hi],
                            in_=ps[:, 0:hi - lo], func=AF.Copy)

        # biases
        rwb_s = keep.tile([128, c.HPT], F32, tag="rwb")
        rrb_s = keep.tile([128, c.HPT], F32, tag="rrb")
        nc.sync.dma_start(out=rwb_s[:], in_=bass.AP(
            tensor=io["rwb"].ap().tensor, offset=0, ap=[[1, 128], [128, c.HPT]]))
        nc.sync.dma_start(out=rrb_s[:], in_=bass.AP(
            tensor=io["rrb"].ap().tensor, offset=0, ap=[[1, 128], [128, c.HPT]]))

        with tc.tile_pool(name="phX", bufs=1) as phX, \
             tc.tile_pool(name="stageX", bufs=3) as stage:
            xT = phX.tile([128, c.DPT * c.KL], BF16, tag="xT")
            load_transposed(io["xw"], xT, stage)
            with tc.tile_pool(name="wpoolX", bufs=1) as wpool:
                f32q = io["qkvw"]
                wq_t, wk_t, wv_t = [], [], []
                for p in range(c.DPT):
                    f32t = stage.tile([128, 3 * c.HD], F32, tag="ldW_qkv")
                    nc.sync.dma_start(out=f32t[:], in_=f32q[ts(p, 128), :])
                    for lst, j, tag in ((wq_t, 0, "wq"), (wk_t, 1, "wk"), (wv_t, 2, "wv")):
                        bt = wpool.tile([128, c.HD], BF16, tag="%s_%d" % (tag, p))
                        nc.vector.tensor_copy(out=bt[:], in_=f32t[:, j * c.HD:(j + 1) * c.HD])
                        lst.append(bt)
                # K^T
                for m in range(c.HPT):
                    for lo, hi in chunks(c.KL):
                        ps = ps_a()
                        for k in range(c.DPT):
                            nc.tensor.matmul(
                                ps[:, 0:hi - lo], wk_t[k][:, ts(m, 128)],
                                xT[:, k * c.KL + lo: k * c.KL + hi],
                                start=(k == 0), stop=(k == c.DPT - 1))
                        nc.scalar.activation(
                            out=kT[:, m * c.KL + lo: m * c.KL + hi],
                            in_=ps[:, 0:hi - lo], func=AF.Copy)
                # V natural (+ ones col per head)
                for m in range(c.NTT):
                    for lo, hi in chunks(c.HD):
                        ps = ps_b()
                        for k in range(c.DPT):
                            nc.tensor.matmul(
                                ps[:, 0:hi - lo], xT[:, k * c.KL + m * 128: k * c.KL + (m + 1) * 128],
                                wv_t[k][:, lo:hi],
                                start=(k == 0), stop=(k == c.DPT - 1))
                        nheads = (hi - lo) // c.DH
                        dst = bass.AP(
                            tensor=vb.tensor,
                            offset=vb.offset + m * VW + (lo // c.DH) * 65,
                            ap=[[c.NTT * VW, 128], [65, nheads], [1, c.DH]])
                        nc.vector.tensor_copy(out=dst, in_=ps[:, 0:hi - lo])
                    ones = bass.AP(
                        tensor=vb.tensor, offset=vb.offset + m * VW + c.DH,
                        ap=[[c.NTT * VW, 128], [65, c.NHC], [1, 1]])
                    nc.vector.memset(ones, 1.0)
                # Q^T with biases
                for m in range(c.HPT):
                    for lo, hi in chunks(c.Q):
                        ps = ps_a()
                        for k in range(c.DPT):
                            nc.tensor.matmul(
                                ps[:, 0:hi - lo], wq_t[k][:, ts(m, 128)],
                                xT[:, k * c.KL + c.M + lo: k * c.KL + c.M + hi],
                                start=(k == 0), stop=(k == c.DPT - 1))
                        sl = slice(m * c.Q + lo, m * c.Q + hi)
                        nc.scalar.activation(out=rwq[:, sl], in_=ps[:, 0:hi - lo],
                                             func=AF.Identity, bias=rwb_s[:, m:m + 1])
                        nc.vector.tensor_scalar_add(out=rrq[:, sl], in0=ps[:, 0:hi - lo],
                                                    scalar1=rrb_s[:, m:m + 1])

        # ============ phase C: attention ============
        with tc.tile_pool(name="score", bufs=2) as score:
            for h in range(c.NHC):
                hp, hr = h // 2, (h % 2) * 64
                pT = score.tile([128, c.NJT * c.Q], BF16, tag="pT")
                for s in range(c.NS):
                    jc, wr_, wst = c.jcomp(s), c.wreal(s), c.wstart(s)
                    bdw = score.tile([128, c.WB], BF16, tag="bdw")
                    for half in range(-(-wr_ // 1024)):
                        lo = half * 1024
                        hi = min(wr_, lo + 1024)
                        ps = ps_a()
                        for nb in range(lo, hi, 512):
                            ne = min(hi, nb + 512)
                            nc.tensor.matmul(
                                ps[:, nb - lo:ne - lo], _qslice(rrq, c, hp, hr, s),
                                rTp[hr:hr + 64, hp * c.WB + wst + nb: hp * c.WB + wst + ne],
                                start=True, stop=True)
                        nc.vector.tensor_scalar_mul(
                            out=bdw[:, lo:hi], in0=ps[:, 0:hi - lo], scalar1=float(c.SCALE))
                    if jc + 128 > wr_:
                        nc.gpsimd.memset(bdw[:, wr_: jc + 128], -30.0)
                    sb = score.tile([128, c.KL], BF16, tag="sb")
                    for half in range(-(-jc // 1024)):
                        lo = half * 1024
                        hi = min(jc, lo + 1024)
                        ps = ps_b()
                        for nb in range(lo, hi, 512):
                            ne = min(hi, nb + 512)
                            nc.tensor.matmul(
                                ps[:, nb - lo:ne - lo], _qslice(rwq, c, hp, hr, s),
                                kT[hr:hr + 64, hp * c.KL + nb: hp * c.KL + ne],
                                start=True, stop=True)
                        nc.scalar.activation(out=sb[:, lo:hi], in_=ps[:, 0:hi - lo],
                                             func=AF.Copy, scale=float(c.SCALE))
                    diag = bass.AP(tensor=bdw.tensor, offset=bdw.offset + 127,
                                   ap=[[c.WB - 1, 128], [1, jc]])
                    nc.gpsimd.dma_start(out=sb[:, 0:jc], in_=diag, accum_op=ALU.add)
                    pb = score.tile([128, c.KL], BF16, tag="pb")
                    nc.scalar.activation(out=pb[:, 0:jc], in_=sb[:, 0:jc], func=AF.Exp)
                    dstap = bass.AP(
                        tensor=pT.tensor, offset=pT.offset + s * 128,
                        ap=[[c.NJT * c.Q, 128], [c.Q, jc // 128], [1, 128]])
                    nc.sync.dma_start(out=dstap, in_=pb[:, 0:jc], transpose=True)
                    if jc < c.KL:
                        z = bass.AP(
                            tensor=pT.tensor,
                            offset=pT.offset + (jc // 128) * c.Q + s * 128,
                            ap=[[c.NJT * c.Q, 128], [c.Q, (c.KL - jc) // 128], [1, 128]])
                        nc.gpsimd.memset(z, 0.0)
                for lo, hi in chunks(c.Q):
                    ps = psB.tile([65, 512], F32, tag="b")
                    for jt in range(c.NJT):
                        nc.tensor.matmul(
                            ps[0:65, 0:hi - lo], vb[:, jt * VW + h * 65: jt * VW + h * 65 + 65],
                            pT[:, jt * c.Q + lo: jt * c.Q + hi],
                            start=(jt == 0), stop=(jt == c.NJT - 1))
                    rd = small.tile([1, 512], F32, tag="rd")
                    nc.vector.reciprocal(out=rd[0:1, 0:hi - lo], in_=ps[64:65, 0:hi - lo])
                    rdb = small.tile([64, 512], F32, tag="rdb")
                    src_b = bass.AP(tensor=rd.tensor, offset=rd.offset,
                                    ap=[[512, 1], [0, 64], [1, hi - lo]])
                    nc.sync.dma_start(out=rdb[:, 0:hi - lo], in_=src_b)
                    nc.vector.tensor_tensor(
                        out=attnT[hr:hr + 64, hp * c.Q + lo: hp * c.Q + hi],
                        in0=ps[0:64, 0:hi - lo], in1=rdb[:, 0:hi - lo], op=ALU.mult)

        attk.release()

        # ============ phase D: o_proj -> natural -> ReduceScatter ============
        with tc.tile_pool(name="wpoolO", bufs=1) as wpool, \
             tc.tile_pool(name="stageD", bufs=3) as stage:
            ow_t = stream_w(io["oww"], c.HPT, c.D, "ow", stage, wpool)
            for m in range(c.DPT):
                for lo, hi in chunks(c.Q):
                    ps = ps_a()
                    for k in range(c.HPT):
                        nc.tensor.matmul(
                            ps[:, 0:hi - lo], ow_t[k][:, ts(m, 128)],
                            attnT[:, k * c.Q + lo: k * c.Q + hi],
                            start=(k == 0), stop=(k == c.HPT - 1))
                    ob = stage.tile([128, 512], F32, tag="oTs")
                    nc.vector.tensor_copy(out=ob[:, 0:hi - lo], in_=ps[:, 0:hi - lo])
                    for q in range((hi - lo) // 128):
                        pst = psB.tile([128, 128], F32, tag="b")
                        nc.tensor.transpose(pst[:], ob[:, ts(q, 128)], identf[:])
                        onat = stage.tile([128, 128], F32, tag="onat")
                        nc.scalar.activation(out=onat[:], in_=pst[:], func=AF.Copy)
                        nc.sync.dma_start(
                            out=io["rs_bin"][ts(lo // 128 + q, 128), ts(m, 128)],
                            in_=onat[:])
        if collective:
            nc.gpsimd.collective_compute(
                "ReduceScatter", ALU.add, replica_groups=rg,
                ins=[io["rs_bin"].ap().opt()], outs=[io["rs_bout"].ap().opt()])
        else:
            # timeline-sim variant: plain copy standing in for the pair RS
            nc.sync.dma_start(out=io["rs_bout"].ap().opt(),
                              in_=io["rs_bin"].ap()[0:c.TOKF, :].opt())
        atp.release()

        # ============ phase E: LN1 + FFN + LN2 ============
        phE = ctx.enter_context(tc.tile_pool(name="phE", bufs=1))
        eps_t = phE.tile([128, 1], F32, tag="eps")
        nc.vector.memset(eps_t[:], c.LN_EPS)
        lns = {}
        for nm in ("ln1g", "ln1b", "ln2g", "ln2b"):
            tl = phE.tile([128, c.D], F32, tag=nm)
            bcast = bass.AP(tensor=io[nm].ap().tensor, offset=0,
                            ap=[[0, 128], [1, c.D]])
            nc.sync.dma_start(out=tl[:], in_=bcast)
            lns[nm] = tl
        fb1 = phE.tile([128, c.DI // 128], F32, tag="fb1")
        nc.sync.dma_start(out=fb1[:], in_=bass.AP(
            tensor=io["ffb1"].ap().tensor, offset=0, ap=[[1, 128], [128, c.DI // 128]]))
        fb2 = phE.tile([128, c.DPT], F32, tag="fb2")
        nc.sync.dma_start(out=fb2[:], in_=bass.AP(
            tensor=io["ffb2"].ap().tensor, offset=0, ap=[[1, 128], [128, c.DPT]]))

        with tc.tile_pool(name="ffn", bufs=1) as ffn, \
             tc.tile_pool(name="stageE", bufs=2) as stage:
            ntt = c.TOKF // 128
            ln1n = ffn.tile([128, ntt * c.D], F32, tag="ln1n")
            lnT = ffn.tile([128, c.DPT * c.TOKF], BF16, tag="lnT")
            for tt in range(ntt):
                z = stage.tile([128, c.D], F32, tag="z")
                nc.sync.dma_start(out=z[:], in_=io["rs_bout"][ts(tt, 128), :])
                wv = stage.tile([128, c.D], F32, tag="wv")
                nc.sync.dma_start(out=wv[:], in_=io["wres"][ts(tt, 128), :])
                nc.vector.tensor_add(out=z[:], in0=z[:], in1=wv[:])
                _layernorm_nat(nc, c, small, z[:], eps_t,
                               lns["ln1g"], lns["ln1b"],
                               ln1n[:, tt * c.D:(tt + 1) * c.D])
                zb = stage.tile([128, c.D], BF16, tag="zb")
                nc.vector.tensor_copy(out=zb[:], in_=ln1n[:, tt * c.D:(tt + 1) * c.D])
                for p in range(c.DPT):
                    pst = psB.tile([128, 128], BF16, tag="b")
                    nc.tensor.transpose(pst[:], zb[:, ts(p, 128)], ident[:])
                    nc.vector.tensor_copy(
                        out=lnT[:, p * c.TOKF + tt * 128: p * c.TOKF + (tt + 1) * 128],
                        in_=pst[:])
            # FFN1: per-m column slice of w1 ([D, 128] -> [128, DPT*128] bf16)
            hT = ffn.tile([128, (c.DI // 128) * c.TOKF], BF16, tag="hT")
            for m in range(c.DI // 128):
                w1f = stage.tile([128, c.DPT * 128], F32, tag="w1f")
                srcap = bass.AP(
                    tensor=io["ffw1"].ap().tensor, offset=m * 128,
                    ap=[[c.DI, 128], [128 * c.DI, c.DPT], [1, 128]])
                nc.sync.dma_start(out=w1f[:], in_=srcap)
                w1m = stage.tile([128, c.DPT * 128], BF16, tag="w1m")
                nc.vector.tensor_copy(out=w1m[:], in_=w1f[:])
                for lo, hi in chunks(c.TOKF):
                    ps = ps_a()
                    for k in range(c.DPT):
                        nc.tensor.matmul(
                            ps[:, 0:hi - lo], w1m[:, ts(k, 128)],
                            lnT[:, k * c.TOKF + lo: k * c.TOKF + hi],
                            start=(k == 0), stop=(k == c.DPT - 1))
                    nc.scalar.activation(
                        out=hT[:, m * c.TOKF + lo: m * c.TOKF + hi],
                        in_=ps[:, 0:hi - lo], func=AF.Relu, bias=fb1[:, m:m + 1])
            # FFN2: per-m column slice of w2 ([DI, 128] -> [128, (DI/128)*128])
            o2T = ffn.tile([128, c.DPT * c.TOKF], F32, tag="o2T")
            nkt = c.DI // 128
            for m in range(c.DPT):
                w2f = stage.tile([128, nkt * 128], F32, tag="w2f")
                srcap = bass.AP(
                    tensor=io["ffw2"].ap().tensor, offset=m * 128,
                    ap=[[c.D, 128], [128 * c.D, nkt], [1, 128]])
                nc.sync.dma_start(out=w2f[:], in_=srcap)
                w2m = stage.tile([128, nkt * 128], BF16, tag="w2m")
                nc.vector.tensor_copy(out=w2m[:], in_=w2f[:])
                for lo, hi in chunks(c.TOKF):
                    ps = ps_a()
                    for k in range(nkt):
                        nc.tensor.matmul(
                            ps[:, 0:hi - lo], w2m[:, ts(k, 128)],
                            hT[:, k * c.TOKF + lo: k * c.TOKF + hi],
                            start=(k == 0), stop=(k == nkt - 1))
                    nc.scalar.activation(
                        out=o2T[:, m * c.TOKF + lo: m * c.TOKF + hi],
                        in_=ps[:, 0:hi - lo], func=AF.Identity, bias=fb2[:, m:m + 1])
            for tt in range(ntt):
                o2n = stage.tile([128, c.D], F32, tag="o2n")
                for p in range(c.DPT):
                    pst = psB.tile([128, 128], F32, tag="b")
                    nc.tensor.transpose(
                        pst[:],
                        o2T[:, p * c.TOKF + tt * 128: p * c.TOKF + (tt + 1) * 128],
                        identf[:])
                    nc.vector.tensor_copy(out=o2n[:, ts(p, 128)], in_=pst[:])
                nc.vector.tensor_add(out=o2n[:], in0=o2n[:],
                                     in1=ln1n[:, tt * c.D:(tt + 1) * c.D])
                fin = stage.tile([128, c.D], F32, tag="fin")
                _layernorm_nat(nc, c, small, o2n[:], eps_t,
                               lns["ln2g"], lns["ln2b"], fin[:])
                nc.sync.dma_start(out=io["out"][ts(tt, 128), :], in_=fin[:])


def _layernorm_nat(nc, c, small, z, eps_t, g, b, out_dst):
    """LayerNorm over the free axis of z [128, D] fp32."""
    BN_FMAX = nc.vector.BN_STATS_FMAX
    d = z.shape[-1]
    sub = math.gcd(BN_FMAX, d)
    nsub = d // sub
    zr = z.rearrange("p (n f) -> p n f", f=sub)
    stats = small.tile([128, nsub, nc.vector.BN_STATS_DIM], F32, tag="bnst")
    for i in range(nsub):
        nc.vector.bn_stats(out=stats[:, i, :], in_=zr[:, i, :])
    mv = small.tile([128, nc.vector.BN_AGGR_DIM], F32, tag="bnag")
    nc.vector.bn_aggr(out=mv[:], in_=stats[:])
    mean, var = mv[:, 0:1], mv[:, 1:2]
    nc.scalar.activation(out=var, in_=var, func=AF.Sqrt, bias=eps_t[:], scale=1.0)
    nc.vector.reciprocal(out=var, in_=var)
    nc.vector.tensor_scalar(out=out_dst, in0=z, scalar1=mean, scalar2=var,
                            op0=ALU.subtract, op1=ALU.mult)
    nc.vector.tensor_tensor(out=out_dst, in0=out_dst, in1=g[:, 0:d], op=ALU.mult)
    nc.vector.tensor_tensor(out=out_dst, in0=out_dst, in1=b[:, 0:d], op=ALU.add)


# ============================================================
# host-side sharding + entry point
# ============================================================

def shard_inputs(inputs, c: Cfg = None):
    c = c or Cfg()
    w = np.asarray(inputs["w"], np.float32)
    r = np.asarray(inputs["r"], np.float32)
    mems = np.asarray(inputs["mems"], np.float32)
    qkv_w = np.asarray(inputs["qkv_w"], np.float32)
    r_net_w = np.asarray(inputs["r_net_w"], np.float32)
    o_w = np.asarray(inputs["o_w"], np.float32)
    r_w_bias = np.asarray(inputs["r_w_bias"], np.float32).reshape(-1)
    r_r_bias = np.asarray(inputs["r_r_bias"], np.float32).reshape(-1)
    NHD = qkv_w.shape[1] // 3
    in_maps = []
    for core in range(c.N_CORES):
        b, hh = core // 2, core % 2
        hsl = slice(hh * c.HD, (hh + 1) * c.HD)
        xw_c = np.concatenate([mems[:, b, :], w[:, b, :]], axis=0)
        qkvw_c = np.concatenate([qkv_w[:, j * NHD + hh * c.HD:
                                       j * NHD + (hh + 1) * c.HD]
                                 for j in range(3)], axis=1)
        in_maps.append({
            "xw": np.ascontiguousarray(xw_c),
            "r_in": np.ascontiguousarray(r[:, 0, :]),
            "qkvw": np.ascontiguousarray(qkvw_c),
            "rnetw": np.ascontiguousarray(r_net_w[:, hsl]),
            "oww": np.ascontiguousarray(o_w[hsl, :]),
            "rwb": np.ascontiguousarray(r_w_bias[hsl][None, :]),
            "rrb": np.ascontiguousarray(r_r_bias[hsl][None, :]),
            "ln1g": np.asarray(inputs["ln1g" if "ln1g" in inputs else "ln1_g"],
                               np.float32).reshape(1, -1),
            "ln1b": np.asarray(inputs["ln1b" if "ln1b" in inputs else "ln1_b"],
                               np.float32).reshape(1, -1),
            "ln2g": np.asarray(inputs["ln2g" if "ln2g" in inputs else "ln2_g"],
                               np.float32).reshape(1, -1),
            "ln2b": np.asarray(inputs["ln2b" if "ln2b" in inputs else "ln2_b"],
                               np.float32).reshape(1, -1),
            "ffw1": np.asarray(inputs["ff_w1"], np.float32),
            "ffb1": np.asarray(inputs["ff_b1"], np.float32).reshape(1, -1),
            "ffw2": np.asarray(inputs["ff_w2"], np.float32),
            "ffb2": np.asarray(inputs["ff_b2"], np.float32).reshape(1, -1),
            "wres": np.ascontiguousarray(w[hh * c.TOKF:(hh + 1) * c.TOKF, b, :]),
        })
    return in_maps


def unshard_output(results, inputs, c: Cfg = None):
    c = c or Cfg()
    w = np.asarray(inputs["w"])
    Q, B, D = w.shape
    out = np.zeros((Q, B, D), np.float32)
    for core in range(c.N_CORES):
        b, hh = core // 2, core % 2
        out[hh * c.TOKF:(hh + 1) * c.TOKF, b, :] = results[core]["out"]
    return out


_NC_CACHE = {}


def kernel(**inputs):
    if "nc" not in _NC_CACHE:
        _NC_CACHE["nc"] = build_kernel()
    nc = _NC_CACHE["nc"]
    in_maps = shard_inputs(inputs)
    from concourse.bass_utils import run_bass_kernel_spmd
    res = run_bass_kernel_spmd(nc, in_maps, core_ids=list(range(Cfg.N_CORES)))
    return unshard_output(res.results, inputs)



# revision 30
# speedup vs baseline: 1.6353x; 1.6353x over previous
"""Trainium2 Bass kernel for nn_MemTransformerLM (Transformer-XL layer).

Sharding (8 cores): batch (4) x head-half (2). Core c handles batch b = c//2
and heads [hh*8, hh*8+8), hh = c%2, for all 1024 queries. After o_proj a
2-rank bf16 ReduceScatter over core pairs (2b, 2b+1) splits tokens for the
FFN: even core keeps tokens [0,512), odd [512,1024).

Key structure:
- All big f32 DRAM inputs are loaded with SWDGE cast-DMA (f32 -> bf16).
- r^T / x^T built via SBUF->SBUF DMA transpose in key-quarters so projection
  matmuls start while later quarters still stream.
- Scores per (head h, q-tile s): BD window matmuls -> PSUM -> drain to bdw
  (window buffer, masked tail memset to -240); AC matmuls -> PSUM -> drain to
  sb; a "diagonal" SBUF->SBUF accum-DMA adds the rel-shifted BD window onto
  sb; one ACT pass computes exp(scale * sb) in place; DMA-transpose into a
  per-s slab; PV accumulates slab tiles over the unmasked j range only.
- o_proj and FFN run in natural token-major orientation (attnT / hT used as
  lhsT), so no fp32 PE transposes and no strided weight loads.
"""

import contextlib
import math

import numpy as np

import concourse.bass as bass
import concourse.bacc as bacc
import concourse.mybir as mybir
import concourse.tile as tile
from concourse.masks import make_identity

F32 = mybir.dt.float32
BF16 = mybir.dt.bfloat16
AF = mybir.ActivationFunctionType
ALU = mybir.AluOpType


class Cfg:
    D = 1024      # model dim
    NHC = 8       # heads per core
    DH = 64       # head dim
    KL = 2048     # key length
    Q = 1024      # query length
    DI = 4096     # ffn inner
    LN_EPS = 1e-5
    N_CORES = 8

    HD = 512          # head dims per core (NHC * DH)
    M = 1024          # mem length
    TOKF = 512        # ffn tokens per core
    WB = 2176         # bdw window buffer width (KL + 128)
    SCALE = 0.125     # 1/sqrt(DH)
    MASKV = -240.0    # pre-scale mask value: exp(SCALE * -240) ~ 1e-13
    DPT = 8           # D / 128
    HPT = 4           # HD / 128
    NTT = 16          # KL / 128
    VW = 520          # NHC * 65

    def jmax(self, s):
        return 1152 + 128 * s

    def wstart(self, s):
        return 896 - 128 * s


def ts(i, n):
    return slice(i * n, (i + 1) * n)


def chunks(total, sz=512):
    return [(lo, min(total, lo + sz)) for lo in range(0, total, sz)]


def build_kernel(c: Cfg = None, collective=True):
    c = c or Cfg()
    nc = bacc.Bacc("TRN2", target_bir_lowering=False)

    io = {}
    def din(name, shape, dt=F32):
        io[name] = nc.dram_tensor(name, shape, dt, kind="ExternalInput")
    din("xw", [c.KL, c.D])
    din("r_in", [c.KL, c.D])
    din("qkvw", [c.D, 3 * c.HD])
    din("rnetw", [c.D, c.HD])
    din("oww", [c.HD, c.D])
    din("rwb", [1, c.HD])
    din("rrb", [1, c.HD])
    din("ln1g", [1, c.D]); din("ln1b", [1, c.D])
    din("ln2g", [1, c.D]); din("ln2b", [1, c.D])
    din("ffw1", [c.D, c.DI]); din("ffb1", [1, c.DI])
    din("ffw2", [c.DI, c.D]); din("ffb2", [1, c.D])
    din("wres", [c.TOKF, c.D])
    io["out"] = nc.dram_tensor("out", [c.TOKF, c.D], F32, kind="ExternalOutput")
    io["rs_bin"] = nc.dram_tensor("rs_bin", [c.Q, c.D], BF16)
    io["rs_bout"] = nc.dram_tensor("rs_bout", [c.TOKF, c.D], BF16)

    with tile.TileContext(nc) as tc:
        _body(tc, nc, c, io, collective=collective)
    nc.finalize()
    return nc


def _body(tc, nc, c, io, collective=True):
    ctx = contextlib.ExitStack()
    rg = [[i, i + 1] for i in range(0, c.N_CORES, 2)]
    with ctx:
        small = ctx.enter_context(tc.tile_pool(name="small", bufs=4))
        psA = ctx.enter_context(tc.tile_pool(name="psA", bufs=3, space="PSUM"))
        psB = ctx.enter_context(tc.tile_pool(name="psB", bufs=3, space="PSUM"))
        psPV = ctx.enter_context(tc.tile_pool(name="psPV", bufs=2, space="PSUM"))

        keep = ctx.enter_context(tc.tile_pool(name="keep", bufs=1))
        ident = keep.tile([128, 128], BF16, tag="identb")
        make_identity(nc, ident)
        identf = keep.tile([128, 128], F32, tag="identf")
        make_identity(nc, identf)

        # biases: rwb/rrb as [128, HPT] (partition = dh within head pair col)
        rwb_s = keep.tile([128, c.HPT], F32, tag="rwb")
        rrb_s = keep.tile([128, c.HPT], F32, tag="rrb")
        nc.sync.dma_start(out=rwb_s[:], in_=bass.AP(
            tensor=io["rwb"].ap().tensor, offset=0, ap=[[1, 128], [128, c.HPT]]))
        nc.sync.dma_start(out=rrb_s[:], in_=bass.AP(
            tensor=io["rrb"].ap().tensor, offset=0, ap=[[1, 128], [128, c.HPT]]))

        # pools created in stack order (release: attk -> atp; rest at exit)
        wo = ctx.enter_context(tc.tile_pool(name="wo", bufs=1))
        wff1 = ctx.enter_context(tc.tile_pool(name="wff1", bufs=1))
        wff2 = ctx.enter_context(tc.tile_pool(name="wff2", bufs=1))
        atp = tc.alloc_tile_pool(name="atp", bufs=1)
        attnT = atp.tile([128, c.HPT * c.Q], BF16, tag="attnT")
        # long-lived attention operands
        attk = tc.alloc_tile_pool(name="attk", bufs=1)
        rTp = attk.tile([128, c.HPT * c.KL], BF16, tag="rTp")
        kT = attk.tile([128, c.HPT * c.KL], BF16, tag="kT")
        vb = attk.tile([128, c.NTT * c.VW], BF16, tag="vb")
        rwq = attk.tile([128, c.HPT * c.Q], BF16, tag="rwq")
        rrq = attk.tile([128, c.HPT * c.Q], BF16, tag="rrq")

        # ============ phase AB: loads + projections, pipelined ============
        with tc.tile_pool(name="phAB", bufs=1) as phAB, \
             tc.tile_pool(name="stAB", bufs=3) as stAB:
            rTq = [phAB.tile([128, c.DPT * 512], BF16, tag="rTq%d" % q,
                             name="rTq%d" % q) for q in range(4)]
            xTq = [phAB.tile([128, c.DPT * 512], BF16, tag="xTq%d" % q,
                             name="xTq%d" % q) for q in range(4)]
            wr = phAB.tile([128, c.DPT * c.HD], BF16, tag="wr")
            wqkv = phAB.tile([128, c.DPT * 3 * c.HD], BF16, tag="wqkv")

            nc.gpsimd.dma_start(out=wr[:], in_=bass.AP(
                tensor=io["rnetw"].ap().tensor, offset=0,
                ap=[[c.HD, 128], [128 * c.HD, c.DPT], [1, c.HD]]))
            nc.gpsimd.dma_start(out=wqkv[:], in_=bass.AP(
                tensor=io["qkvw"].ap().tensor, offset=0,
                ap=[[3 * c.HD, 128], [128 * 3 * c.HD, c.DPT], [1, 3 * c.HD]]))

            def load_transposed(src, dstq):
                # 8 cast-DMAs of 2 key-tiles each; transpose per key-tile
                for g in range(8):
                    nat = stAB.tile([128, 2 * c.D], BF16, tag="nat")
                    nc.gpsimd.dma_start(out=nat[:], in_=bass.AP(
                        tensor=src.ap().tensor, offset=g * 2 * 128 * c.D,
                        ap=[[c.D, 128], [128 * c.D, 2], [1, c.D]]))
                    for h2 in range(2):
                        tt = g * 2 + h2
                        dst = dstq[tt // 4]
                        dstap = bass.AP(
                            tensor=dst.tensor,
                            offset=dst.offset + (tt % 4) * 128,
                            ap=[[c.DPT * 512, 128], [512, c.DPT], [1, 128]])
                        nc.sync.dma_start(
                            out=dstap, in_=nat[:, ts(h2, c.D)], transpose=True)

            load_transposed(io["r_in"], rTq)
            load_transposed(io["xw"], xTq)

            # rTp: r_head_k^T per head-pair m
            for m in range(c.HPT):
                for q4 in range(4):
                    ps = psA.tile([128, 512], F32, tag="a", name="psa")
                    for k in range(c.DPT):
                        nc.tensor.matmul(
                            ps[:], wr[:, k * c.HD + m * 128: k * c.HD + (m + 1) * 128],
                            rTq[q4][:, ts(k, 512)],
                            start=(k == 0), stop=(k == c.DPT - 1))
                    nc.vector.tensor_copy(
                        out=rTp[:, m * c.KL + q4 * 512: m * c.KL + (q4 + 1) * 512],
                        in_=ps[:])
            # K^T
            for m in range(c.HPT):
                for q4 in range(4):
                    ps = psB.tile([128, 512], F32, tag="b", name="psb")
                    for k in range(c.DPT):
                        nc.tensor.matmul(
                            ps[:], wqkv[:, k * 1536 + c.HD + m * 128:
                                        k * 1536 + c.HD + (m + 1) * 128],
                            xTq[q4][:, ts(k, 512)],
                            start=(k == 0), stop=(k == c.DPT - 1))
                    nc.scalar.activation(
                        out=kT[:, m * c.KL + q4 * 512: m * c.KL + (q4 + 1) * 512],
                        in_=ps[:], func=AF.Copy)
            # Q^T with biases (queries = keys [M, KL) = quarters 2,3)
            for m in range(c.HPT):
                for qc in range(2):
                    ps = psA.tile([128, 512], F32, tag="a", name="psa")
                    for k in range(c.DPT):
                        nc.tensor.matmul(
                            ps[:], wqkv[:, k * 1536 + m * 128: k * 1536 + (m + 1) * 128],
                            xTq[2 + qc][:, ts(k, 512)],
                            start=(k == 0), stop=(k == c.DPT - 1))
                    sl = slice(m * c.Q + qc * 512, m * c.Q + (qc + 1) * 512)
                    nc.scalar.activation(out=rwq[:, sl], in_=ps[:],
                                         func=AF.Identity, bias=rwb_s[:, m:m + 1])
                    nc.vector.tensor_scalar_add(out=rrq[:, sl], in0=ps[:],
                                                scalar1=rrb_s[:, m:m + 1])
            # V natural (+ ones col per head)
            for jt in range(c.NTT):
                ps = psC.tile([128, 512], F32, tag="c", name="psc")
                for k in range(c.DPT):
                    nc.tensor.matmul(
                        ps[:], xTq[jt // 4][:, k * 512 + (jt % 4) * 128:
                                            k * 512 + (jt % 4 + 1) * 128],
                        wqkv[:, k * 1536 + 2 * c.HD: k * 1536 + 3 * c.HD],
                        start=(k == 0), stop=(k == c.DPT - 1))
                dst = bass.AP(
                    tensor=vb.tensor, offset=vb.offset + jt * c.VW,
                    ap=[[c.NTT * c.VW, 128], [65, c.NHC], [1, c.DH]])
                nc.vector.tensor_copy(out=dst, in_=ps[:])
                ones = bass.AP(
                    tensor=vb.tensor, offset=vb.offset + jt * c.VW + c.DH,
                    ap=[[c.NTT * c.VW, 128], [65, c.NHC], [1, 1]])
                nc.vector.memset(ones, 1.0)

        # ============ phase C: attention ============
        w1h = wff1.tile([128, c.DPT * 2048], BF16, tag="w1h")

        def drain(eng_i, out_ap, in_ap):
            if eng_i == 0:
                nc.vector.tensor_copy(out=out_ap, in_=in_ap)
            elif eng_i == 1:
                nc.scalar.activation(out=out_ap, in_=in_ap, func=AF.Copy)
            else:
                n = in_ap.shape[-1]
                h = (n * 5 // 8) & ~63 or n // 2
                nc.vector.tensor_copy(out=out_ap[:, 0:h], in_=in_ap[:, 0:h])
                nc.scalar.activation(out=out_ap[:, h:n], in_=in_ap[:, h:n],
                                     func=AF.Copy)

        BD_ENG = [0, 1, 0, 1]   # DVE, ACT, ...
        AC_ENG = [1, 0, 1, 0]

        with tc.tile_pool(name="score", bufs=4) as score, \
             tc.tile_pool(name="slabp", bufs=5) as slabp:
            owb = wo.tile([128, c.HPT * c.D], BF16, tag="owb")
            nc.gpsimd.dma_start(out=owb[:], in_=bass.AP(
                tensor=io["oww"].ap().tensor, offset=0,
                ap=[[c.D, 128], [128 * c.D, c.HPT], [1, c.D]]))

            pvs = [None, None]
            exp_q = []
            pv_q = []

            def emit_exp(st):
                sb, bdw, jmax = st["sb"], st["bdw"], st["jmax"]
                diag = bass.AP(tensor=bdw.tensor, offset=bdw.offset + 127,
                               ap=[[c.WB - 1, 128], [1, jmax]])
                nc.gpsimd.dma_start(out=sb[:, 0:jmax], in_=diag,
                                    accum_op=ALU.add)
                slab = slabp.tile([128, c.KL], BF16, tag="slab", name="slab")
                for hlo, hhi in ((0, 1024), (1024, jmax)):
                    nc.scalar.activation(out=sb[:, hlo:hhi], in_=sb[:, hlo:hhi],
                                         func=AF.Exp, scale=float(c.SCALE))
                    dstap = bass.AP(
                        tensor=slab.tensor, offset=slab.offset + hlo,
                        ap=[[c.KL, 128], [128, (hhi - hlo) // 128], [1, 128]])
                    nc.sync.dma_start(out=dstap, in_=sb[:, hlo:hhi],
                                      transpose=True)
                st["slab"] = slab

            def emit_pv(st):
                h, s, jmax, slab = st["h"], st["s"], st["jmax"], st["slab"]
                hp, hr = h // 2, (h % 2) * 64
                njt = jmax // 128
                if s % 4 == 0:
                    pv = pvs[h % 2] = psPV.tile([65, 512], F32, tag="pv",
                                                name="pspv")
                else:
                    pv = pvs[h % 2]
                col = (s % 4) * 128
                for jt in range(njt):
                    nc.tensor.matmul(
                        pv[0:65, col:col + 128],
                        vb[:, jt * c.VW + h * 65: jt * c.VW + h * 65 + 65],
                        slab[:, ts(jt, 128)],
                        start=(jt == 0), stop=(jt == njt - 1))
                if s % 4 == 3:
                    g = s // 4
                    rd = small.tile([1, 512], F32, tag="rd")
                    nc.vector.reciprocal(out=rd[0:1, :], in_=pv[64:65, :])
                    rdb = small.tile([64, 512], F32, tag="rdb")
                    src_b = bass.AP(tensor=rd.tensor, offset=rd.offset,
                                    ap=[[512, 1], [0, 64], [1, 512]])
                    nc.sync.dma_start(out=rdb[:], in_=src_b)
                    nc.vector.tensor_tensor(
                        out=attnT[hr:hr + 64, hp * c.Q + g * 512:
                                  hp * c.Q + (g + 1) * 512],
                        in0=pv[0:64, :], in1=rdb[:], op=ALU.mult)

            for hp in range(c.HPT):
                for s in range(8):
                  for h2 in range(2):
                    h = 2 * hp + h2
                    hr = h2 * 64
                    jmax, wst = c.jmax(s), c.wstart(s)
                    # --- BD window ---
                    bdw = score.tile([128, c.WB], BF16, tag="bdw")
                    for ci, (lo, hi) in enumerate(chunks(jmax)):
                        ps = psB.tile([128, 512], F32, tag="b", name="psbd")
                        nc.tensor.matmul(
                            ps[:, 0:hi - lo],
                            rrq[hr:hr + 64, hp * c.Q + s * 128: hp * c.Q + (s + 1) * 128],
                            rTp[hr:hr + 64, hp * c.KL + wst + lo: hp * c.KL + wst + hi],
                            start=True, stop=True)
                        drain(BD_ENG[h % 2], bdw[:, lo:hi], ps[:, 0:hi - lo])
                    nc.gpsimd.memset(bdw[:, jmax:jmax + 128], c.MASKV)
                    # --- AC ---
                    sb = score.tile([128, c.KL], BF16, tag="sb")
                    for lo, hi in chunks(jmax):
                        ps = psA.tile([128, 512], F32, tag="a", name="psac")
                        nc.tensor.matmul(
                            ps[:, 0:hi - lo],
                            rwq[hr:hr + 64, hp * c.Q + s * 128: hp * c.Q + (s + 1) * 128],
                            kT[hr:hr + 64, hp * c.KL + lo: hp * c.KL + hi],
                            start=True, stop=True)
                        drain(AC_ENG[h % 2], sb[:, lo:hi], ps[:, 0:hi - lo])
                    st = {"h": h, "s": s, "jmax": jmax, "sb": sb, "bdw": bdw}
                    exp_q.append(st)
                    pv_q.append(st)
                    if len(exp_q) > 1:
                        emit_exp(exp_q.pop(0))
                    if len(pv_q) > 2:
                        emit_pv(pv_q.pop(0))
            while exp_q:
                emit_exp(exp_q.pop(0))
            while pv_q:
                emit_pv(pv_q.pop(0))

        attk.release()

        # ============ phase D: o_proj (natural) -> ReduceScatter ============
        with tc.tile_pool(name="stD", bufs=3) as stD:
            for t in range(8):
                ob = stD.tile([128, c.D], BF16, tag="ob")
                for ci in range(2):
                    ps = psA.tile([128, 512], F32, tag="a", name="pso")
                    for k in range(c.HPT):
                        nc.tensor.matmul(
                            ps[:], attnT[:, k * c.Q + t * 128: k * c.Q + (t + 1) * 128],
                            owb[:, k * c.D + ci * 512: k * c.D + (ci + 1) * 512],
                            start=(k == 0), stop=(k == c.HPT - 1))
                    nc.scalar.activation(out=ob[:, ts(ci, 512)], in_=ps[:],
                                         func=AF.Copy)
                nc.sync.dma_start(out=io["rs_bin"][ts(t, 128), :], in_=ob[:])
            if collective:
                nc.gpsimd.collective_compute(
                    "ReduceScatter", ALU.add, replica_groups=rg,
                    ins=[io["rs_bin"].ap().opt()], outs=[io["rs_bout"].ap().opt()])
            else:
                nc.sync.dma_start(out=io["rs_bout"].ap().opt(),
                                  in_=io["rs_bin"].ap()[0:c.TOKF, :].opt())

            # w2 cast-load (during collective window)
            w2t = wff2.tile([128, 32 * c.D], BF16, tag="w2t")
            for g in range(4):
                nc.gpsimd.dma_start(
                    out=w2t[:, g * 8 * c.D:(g + 1) * 8 * c.D],
                    in_=bass.AP(tensor=io["ffw2"].ap().tensor,
                                offset=g * 8 * 128 * c.D,
                                ap=[[c.D, 128], [128 * c.D, 8], [1, c.D]]))
        atp.release()

        # ============ phase E: LN1 + FFN + LN2 ============
        phE = ctx.enter_context(tc.tile_pool(name="phE", bufs=1))
        eps_t = phE.tile([128, 1], F32, tag="eps")
        nc.vector.memset(eps_t[:], c.LN_EPS)
        lns = {}
        for nm in ("ln1g", "ln1b", "ln2g", "ln2b", "ffb2"):
            tl = phE.tile([128, c.D], BF16, tag=nm)
            bcast = bass.AP(tensor=io[nm].ap().tensor, offset=0,
                            ap=[[0, 128], [1, c.D]])
            nc.gpsimd.dma_start(out=tl[:], in_=bcast)
            lns[nm] = tl
        fb1 = phE.tile([128, c.DI // 128], F32, tag="fb1")
        nc.sync.dma_start(out=fb1[:], in_=bass.AP(
            tensor=io["ffb1"].ap().tensor, offset=0,
            ap=[[1, 128], [128, c.DI // 128]]))

        ffn = ctx.enter_context(tc.tile_pool(name="ffn", bufs=1))
        ntt = c.TOKF // 128
        ln1n = ffn.tile([128, ntt * c.D], F32, tag="ln1n")
        lnT = ffn.tile([128, c.DPT * c.TOKF], BF16, tag="lnT")
        hT = ffn.tile([128, 32 * c.TOKF], BF16, tag="hT")

        with tc.tile_pool(name="stE1", bufs=2) as stE1:
            for t in range(ntt):
                zt = stE1.tile([128, c.D], BF16, tag="zt")
                nc.sync.dma_start(out=zt[:], in_=io["rs_bout"][ts(t, 128), :])
                wv = stE1.tile([128, c.D], F32, tag="wv")
                nc.sync.dma_start(out=wv[:], in_=io["wres"][ts(t, 128), :])
                zf = stE1.tile([128, c.D], F32, tag="zf")
                nc.vector.tensor_tensor(out=zf[:], in0=wv[:], in1=zt[:],
                                        op=ALU.add)
                _layernorm_nat(nc, c, small, zf[:], eps_t,
                               lns["ln1g"], lns["ln1b"], ln1n[:, ts(t, c.D)])
                for g in range(2):
                    pst = psB.tile([128, 512], F32, tag="b", name="pstr")
                    for j in range(4):
                        nc.tensor.transpose(
                            pst[:, ts(j, 128)],
                            ln1n[:, t * c.D + (g * 4 + j) * 128:
                                 t * c.D + (g * 4 + j + 1) * 128],
                            identf[:])
                    dst = bass.AP(
                        tensor=lnT.tensor,
                        offset=lnT.offset + g * 4 * c.TOKF + t * 128,
                        ap=[[c.DPT * c.TOKF, 128], [c.TOKF, 4], [1, 128]])
                    nc.vector.tensor_copy(out=dst, in_=pst[:])

        def load_w2g(g):
            w2g = wff2.tile([128, 8 * c.D], BF16, tag="w2g", name="w2g")
            nc.gpsimd.dma_start(
                out=w2g[:],
                in_=bass.AP(tensor=io["ffw2"].ap().tensor,
                            offset=g * 8 * 128 * c.D,
                            ap=[[c.D, 128], [128 * c.D, 8], [1, c.D]]))
            return w2g

        w2cache = {}
        # FFN1: hT[di, tok] = relu(w1^T @ ln1^T + b1); w1 streamed in quarters
        for quarter in range(4):
            w1q = w1q_cur
            if quarter < 3:
                w1q_cur = load_w1q(quarter + 1)
            if quarter >= 2:
                g = quarter - 2
                w2cache[g] = load_w2g(g)
            for mm in range(8):
                m = quarter * 8 + mm
                pp, tg = (psA, "a") if mm % 2 == 0 else (psB, "b")
                ps = pp.tile([128, 512], F32, tag=tg, name="psf1")
                for k in range(c.DPT):
                    nc.tensor.matmul(
                        ps[:], w1q[:, k * 1024 + mm * 128: k * 1024 + (mm + 1) * 128],
                        lnT[:, ts(k, c.TOKF)],
                        start=(k == 0), stop=(k == c.DPT - 1))
                nc.scalar.activation(
                    out=hT[:, ts(m, c.TOKF)], in_=ps[:],
                    func=AF.Relu, bias=fb1[:, m:m + 1])

        # FFN2: natural out = hT^T @ w2 (+ residual + b2), LN2
        with tc.tile_pool(name="stE2", bufs=2) as stE2:
            for t in range(ntt):
                o2n = stE2.tile([128, c.D], F32, tag="o2n")
                for ci in range(2):
                    pp, tg = (psA, "a") if ci == 0 else (psC, "c")
                    ps = pp.tile([128, 512], F32, tag=tg, name="psf2")
                    for m in range(32):
                        nc.tensor.matmul(
                            ps[:], hT[:, m * c.TOKF + t * 128: m * c.TOKF + (t + 1) * 128],
                            w2t[:, m * c.D + ci * 512: m * c.D + (ci + 1) * 512],
                            start=(m == 0), stop=(m == 31))
                    nc.vector.tensor_tensor(
                        out=o2n[:, ts(ci, 512)], in0=ps[:],
                        in1=ln1n[:, t * c.D + ci * 512: t * c.D + (ci + 1) * 512],
                        op=ALU.add)
                nc.vector.tensor_tensor(out=o2n[:], in0=o2n[:],
                                        in1=lns["ffb2"][:], op=ALU.add)
                fin = stE2.tile([128, c.D], F32, tag="fin")
                _layernorm_nat(nc, c, small, o2n[:], eps_t,
                               lns["ln2g"], lns["ln2b"], fin[:])
                nc.sync.dma_start(out=io["out"][ts(t, 128), :], in_=fin[:])


def _layernorm_nat(nc, c, small, z, eps_t, g, b, out_dst):
    """LayerNorm over the free axis of z [128, D] fp32."""
    BN_FMAX = nc.vector.BN_STATS_FMAX
    d = z.shape[-1]
    sub = math.gcd(BN_FMAX, d)
    nsub = d // sub
    zr = z.rearrange("p (n f) -> p n f", f=sub)
    stats = small.tile([128, nsub, nc.vector.BN_STATS_DIM], F32, tag="bnst")
    for i in range(nsub):
        nc.vector.bn_stats(out=stats[:, i, :], in_=zr[:, i, :])
    mv = small.tile([128, nc.vector.BN_AGGR_DIM], F32, tag="bnag")
    nc.vector.bn_aggr(out=mv[:], in_=stats[:])
    mean, var = mv[:, 0:1], mv[:, 1:2]
    nc.scalar.activation(out=var, in_=var, func=AF.Sqrt, bias=eps_t[:], scale=1.0)
    nc.vector.reciprocal(out=var, in_=var)
    nc.vector.tensor_scalar(out=out_dst, in0=z, scalar1=mean, scalar2=var,
                            op0=ALU.subtract, op1=ALU.mult)
    nc.vector.tensor_tensor(out=out_dst, in0=out_dst, in1=g[:, 0:d], op=ALU.mult)
    nc.vector.tensor_tensor(out=out_dst, in0=out_dst, in1=b[:, 0:d], op=ALU.add)


# ============================================================
# host-side sharding + entry point
# ============================================================

def shard_inputs(inputs, c: Cfg = None):
    c = c or Cfg()
    w = np.asarray(inputs["w"], np.float32)
    r = np.asarray(inputs["r"], np.float32)
    mems = np.asarray(inputs["mems"], np.float32)
    qkv_w = np.asarray(inputs["qkv_w"], np.float32)
    r_net_w = np.asarray(inputs["r_net_w"], np.float32)
    o_w = np.asarray(inputs["o_w"], np.float32)
    r_w_bias = np.asarray(inputs["r_w_bias"], np.float32).reshape(-1)
    r_r_bias = np.asarray(inputs["r_r_bias"], np.float32).reshape(-1)
    NHD = qkv_w.shape[1] // 3
    in_maps = []
    for core in range(c.N_CORES):
        b, hh = core // 2, core % 2
        hsl = slice(hh * c.HD, (hh + 1) * c.HD)
        xw_c = np.concatenate([mems[:, b, :], w[:, b, :]], axis=0)
        qkvw_c = np.concatenate([qkv_w[:, j * NHD + hh * c.HD:
                                       j * NHD + (hh + 1) * c.HD]
                                 for j in range(3)], axis=1)
        in_maps.append({
            "xw": np.ascontiguousarray(xw_c),
            "r_in": np.ascontiguousarray(r[:, 0, :]),
            "qkvw": np.ascontiguousarray(qkvw_c),
            "rnetw": np.ascontiguousarray(r_net_w[:, hsl]),
            "oww": np.ascontiguousarray(o_w[hsl, :]),
            "rwb": np.ascontiguousarray(r_w_bias[hsl][None, :]),
            "rrb": np.ascontiguousarray(r_r_bias[hsl][None, :]),
            "ln1g": np.asarray(inputs["ln1_g"], np.float32).reshape(1, -1),
            "ln1b": np.asarray(inputs["ln1_b"], np.float32).reshape(1, -1),
            "ln2g": np.asarray(inputs["ln2_g"], np.float32).reshape(1, -1),
            "ln2b": np.asarray(inputs["ln2_b"], np.float32).reshape(1, -1),
            "ffw1": np.asarray(inputs["ff_w1"], np.float32),
            "ffb1": np.asarray(inputs["ff_b1"], np.float32).reshape(1, -1),
            "ffw2": np.asarray(inputs["ff_w2"], np.float32),
            "ffb2": np.asarray(inputs["ff_b2"], np.float32).reshape(1, -1),
            "wres": np.ascontiguousarray(w[hh * c.TOKF:(hh + 1) * c.TOKF, b, :]),
        })
    return in_maps


def unshard_output(results, inputs, c: Cfg = None):
    c = c or Cfg()
    w = np.asarray(inputs["w"])
    Q, B, D = w.shape
    out = np.zeros((Q, B, D), np.float32)
    for core in range(c.N_CORES):
        b, hh = core // 2, core % 2
        out[hh * c.TOKF:(hh + 1) * c.TOKF, b, :] = results[core]["out"]
    return out


_NC_CACHE = {}


def kernel(**inputs):
    if "nc" not in _NC_CACHE:
        _NC_CACHE["nc"] = build_kernel()
    nc = _NC_CACHE["nc"]
    in_maps = shard_inputs(inputs)
    from concourse.bass_utils import run_bass_kernel_spmd
    res = run_bass_kernel_spmd(nc, in_maps, core_ids=list(range(Cfg.N_CORES)))
    return unshard_output(res.results, inputs)


# revision 41
# speedup vs baseline: 1.6618x; 1.0162x over previous
"""Trainium2 Bass kernel for nn_MemTransformerLM (Transformer-XL layer).

Sharding (8 cores): batch (4) x head-half (2). Core c handles batch b = c//2
and heads [hh*8, hh*8+8), hh = c%2, for all 1024 queries. After o_proj a
2-rank bf16 ReduceScatter over core pairs (2b, 2b+1) splits tokens for the
FFN: even core keeps tokens [0,512), odd [512,1024).

Key structure:
- All big f32 DRAM inputs are loaded with SWDGE cast-DMA (f32 -> bf16).
- r^T / x^T built via SBUF->SBUF DMA transpose in key-quarters so projection
  matmuls start while later quarters still stream.
- Scores per (head h, q-tile s): BD window matmuls -> PSUM -> drain to bdw
  (window buffer, masked tail memset to -240); AC matmuls -> PSUM -> drain to
  sb; a "diagonal" SBUF->SBUF accum-DMA adds the rel-shifted BD window onto
  sb; one ACT pass computes exp(scale * sb) in place; DMA-transpose into a
  per-s slab; PV accumulates slab tiles over the unmasked j range only.
- o_proj and FFN run in natural token-major orientation (attnT / hT used as
  lhsT), so no fp32 PE transposes and no strided weight loads.
"""

import contextlib
import math

import numpy as np

import concourse.bass as bass
import concourse.bacc as bacc
import concourse.mybir as mybir
import concourse.tile as tile
from concourse.masks import make_identity

F32 = mybir.dt.float32
BF16 = mybir.dt.bfloat16
AF = mybir.ActivationFunctionType
ALU = mybir.AluOpType


class Cfg:
    D = 1024      # model dim
    NHC = 8       # heads per core
    DH = 64       # head dim
    KL = 2048     # key length
    Q = 1024      # query length
    DI = 4096     # ffn inner
    LN_EPS = 1e-5
    N_CORES = 8

    HD = 512          # head dims per core (NHC * DH)
    M = 1024          # mem length
    TOKF = 512        # ffn tokens per core
    WB = 2176         # bdw window buffer width (KL + 128)
    SCALE = 0.125     # 1/sqrt(DH)
    MASKV = -240.0    # pre-scale mask value: exp(SCALE * -240) ~ 1e-13
    DPT = 8           # D / 128
    HPT = 4           # HD / 128
    NTT = 16          # KL / 128
    VW = 520          # NHC * 65

    def jmax(self, s):
        return 1152 + 128 * s

    def wstart(self, s):
        return 896 - 128 * s


def ts(i, n):
    return slice(i * n, (i + 1) * n)


def chunks(total, sz=512):
    return [(lo, min(total, lo + sz)) for lo in range(0, total, sz)]


def build_kernel(c: Cfg = None, collective=True):
    c = c or Cfg()
    nc = bacc.Bacc("TRN2", target_bir_lowering=False)

    io = {}
    def din(name, shape, dt=F32):
        io[name] = nc.dram_tensor(name, shape, dt, kind="ExternalInput")
    din("xw", [c.KL, c.D])
    din("r_in", [c.KL, c.D])
    din("qkvw", [c.D, 3 * c.HD])
    din("rnetw", [c.D, c.HD])
    din("oww", [c.HD, c.D])
    din("rwb", [1, c.HD])
    din("rrb", [1, c.HD])
    din("ln1g", [1, c.D]); din("ln1b", [1, c.D])
    din("ln2g", [1, c.D]); din("ln2b", [1, c.D])
    din("ffw1", [c.D, c.DI]); din("ffb1", [1, c.DI])
    din("ffw2", [c.DI, c.D]); din("ffb2", [1, c.D])
    din("wres", [c.TOKF, c.D])
    io["out"] = nc.dram_tensor("out", [c.TOKF, c.D], F32, kind="ExternalOutput")
    io["rs_bin"] = nc.dram_tensor("rs_bin", [c.Q, c.D], BF16)
    io["rs_bout"] = nc.dram_tensor("rs_bout", [c.TOKF, c.D], BF16)

    with tile.TileContext(nc) as tc:
        _body(tc, nc, c, io, collective=collective)
    nc.finalize()
    return nc


def _body(tc, nc, c, io, collective=True):
    ctx = contextlib.ExitStack()
    rg = [[i, i + 1] for i in range(0, c.N_CORES, 2)]
    with ctx:
        small = ctx.enter_context(tc.tile_pool(name="small", bufs=4))
        psA = ctx.enter_context(tc.tile_pool(name="psA", bufs=3, space="PSUM"))
        psB = ctx.enter_context(tc.tile_pool(name="psB", bufs=3, space="PSUM"))
        psPV = ctx.enter_context(tc.tile_pool(name="psPV", bufs=2, space="PSUM"))

        keep = ctx.enter_context(tc.tile_pool(name="keep", bufs=1))
        ident = keep.tile([128, 128], BF16, tag="identb")
        make_identity(nc, ident)
        identf = keep.tile([128, 128], F32, tag="identf")
        make_identity(nc, identf)

        # biases: rwb/rrb as [128, HPT] (partition = dh within head pair col)
        rwb_s = keep.tile([128, c.HPT], F32, tag="rwb")
        rrb_s = keep.tile([128, c.HPT], F32, tag="rrb")
        nc.sync.dma_start(out=rwb_s[:], in_=bass.AP(
            tensor=io["rwb"].ap().tensor, offset=0, ap=[[1, 128], [128, c.HPT]]))
        nc.sync.dma_start(out=rrb_s[:], in_=bass.AP(
            tensor=io["rrb"].ap().tensor, offset=0, ap=[[1, 128], [128, c.HPT]]))

        # pools created in stack order (release: attk -> atp; rest at exit)
        wo = ctx.enter_context(tc.tile_pool(name="wo", bufs=1))
        wff1 = ctx.enter_context(tc.tile_pool(name="wff1", bufs=1))
        wff2 = ctx.enter_context(tc.tile_pool(name="wff2", bufs=1))
        atp = tc.alloc_tile_pool(name="atp", bufs=1)
        attnT = atp.tile([128, c.HPT * c.Q], BF16, tag="attnT")
        # long-lived attention operands
        attk = tc.alloc_tile_pool(name="attk", bufs=1)
        rTp = attk.tile([128, c.HPT * c.KL], BF16, tag="rTp")
        kT = attk.tile([128, c.HPT * c.KL], BF16, tag="kT")
        vb = attk.tile([128, c.NTT * c.VW], BF16, tag="vb")
        rwq = attk.tile([128, c.HPT * c.Q], BF16, tag="rwq")
        rrq = attk.tile([128, c.HPT * c.Q], BF16, tag="rrq")

        # ============ phase AB: loads + projections, pipelined ============
        with tc.tile_pool(name="phAB", bufs=1) as phAB, \
             tc.tile_pool(name="stAB", bufs=3) as stAB:
            rTq = [phAB.tile([128, c.DPT * 512], BF16, tag="rTq%d" % q,
                             name="rTq%d" % q) for q in range(4)]
            xTq = [phAB.tile([128, c.DPT * 512], BF16, tag="xTq%d" % q,
                             name="xTq%d" % q) for q in range(4)]
            wr = phAB.tile([128, c.DPT * c.HD], BF16, tag="wr")
            wqkv = phAB.tile([128, c.DPT * 3 * c.HD], BF16, tag="wqkv")

            nc.gpsimd.dma_start(out=wr[:], in_=bass.AP(
                tensor=io["rnetw"].ap().tensor, offset=0,
                ap=[[c.HD, 128], [128 * c.HD, c.DPT], [1, c.HD]]))
            nc.gpsimd.dma_start(out=wqkv[:], in_=bass.AP(
                tensor=io["qkvw"].ap().tensor, offset=0,
                ap=[[3 * c.HD, 128], [128 * 3 * c.HD, c.DPT], [1, 3 * c.HD]]))

            def load_transposed(src, dstq):
                # 8 cast-DMAs of 2 key-tiles each; transpose per key-tile
                for g in range(8):
                    nat = stAB.tile([128, 2 * c.D], BF16, tag="nat")
                    nc.gpsimd.dma_start(out=nat[:], in_=bass.AP(
                        tensor=src.ap().tensor, offset=g * 2 * 128 * c.D,
                        ap=[[c.D, 128], [128 * c.D, 2], [1, c.D]]))
                    for h2 in range(2):
                        tt = g * 2 + h2
                        dst = dstq[tt // 4]
                        dstap = bass.AP(
                            tensor=dst.tensor,
                            offset=dst.offset + (tt % 4) * 128,
                            ap=[[c.DPT * 512, 128], [512, c.DPT], [1, 128]])
                        nc.sync.dma_start(
                            out=dstap, in_=nat[:, ts(h2, c.D)], transpose=True)

            load_transposed(io["r_in"], rTq)
            load_transposed(io["xw"], xTq)

            # rTp: r_head_k^T per head-pair m
            for m in range(c.HPT):
                for q4 in range(4):
                    ps = psA.tile([128, 512], F32, tag="a", name="psa")
                    for k in range(c.DPT):
                        nc.tensor.matmul(
                            ps[:], wr[:, k * c.HD + m * 128: k * c.HD + (m + 1) * 128],
                            rTq[q4][:, ts(k, 512)],
                            start=(k == 0), stop=(k == c.DPT - 1))
                    nc.vector.tensor_copy(
                        out=rTp[:, m * c.KL + q4 * 512: m * c.KL + (q4 + 1) * 512],
                        in_=ps[:])
            # K^T
            for m in range(c.HPT):
                for q4 in range(4):
                    ps = psB.tile([128, 512], F32, tag="b", name="psb")
                    for k in range(c.DPT):
                        nc.tensor.matmul(
                            ps[:], wqkv[:, k * 1536 + c.HD + m * 128:
                                        k * 1536 + c.HD + (m + 1) * 128],
                            xTq[q4][:, ts(k, 512)],
                            start=(k == 0), stop=(k == c.DPT - 1))
                    nc.scalar.activation(
                        out=kT[:, m * c.KL + q4 * 512: m * c.KL + (q4 + 1) * 512],
                        in_=ps[:], func=AF.Copy)
            # Q^T with biases (queries = keys [M, KL) = quarters 2,3)
            for m in range(c.HPT):
                for qc in range(2):
                    ps = psA.tile([128, 512], F32, tag="a", name="psa")
                    for k in range(c.DPT):
                        nc.tensor.matmul(
                            ps[:], wqkv[:, k * 1536 + m * 128: k * 1536 + (m + 1) * 128],
                            xTq[2 + qc][:, ts(k, 512)],
                            start=(k == 0), stop=(k == c.DPT - 1))
                    sl = slice(m * c.Q + qc * 512, m * c.Q + (qc + 1) * 512)
                    nc.scalar.activation(out=rwq[:, sl], in_=ps[:],
                                         func=AF.Identity, bias=rwb_s[:, m:m + 1])
                    nc.vector.tensor_scalar_add(out=rrq[:, sl], in0=ps[:],
                                                scalar1=rrb_s[:, m:m + 1])
            # V natural (+ ones col per head)
            for jt in range(c.NTT):
                ps = psC.tile([128, 512], F32, tag="c", name="psc")
                for k in range(c.DPT):
                    nc.tensor.matmul(
                        ps[:], xTq[jt // 4][:, k * 512 + (jt % 4) * 128:
                                            k * 512 + (jt % 4 + 1) * 128],
                        wqkv[:, k * 1536 + 2 * c.HD: k * 1536 + 3 * c.HD],
                        start=(k == 0), stop=(k == c.DPT - 1))
                dst = bass.AP(
                    tensor=vb.tensor, offset=vb.offset + jt * c.VW,
                    ap=[[c.NTT * c.VW, 128], [65, c.NHC], [1, c.DH]])
                nc.vector.tensor_copy(out=dst, in_=ps[:])
                ones = bass.AP(
                    tensor=vb.tensor, offset=vb.offset + jt * c.VW + c.DH,
                    ap=[[c.NTT * c.VW, 128], [65, c.NHC], [1, 1]])
                nc.vector.memset(ones, 1.0)

        # ============ phase C: attention ============
        w1h = wff1.tile([128, c.DPT * 2048], BF16, tag="w1h")

        def drain(eng_i, out_ap, in_ap):
            if eng_i == 0:
                nc.vector.tensor_copy(out=out_ap, in_=in_ap)
            elif eng_i == 1:
                nc.scalar.activation(out=out_ap, in_=in_ap, func=AF.Copy)
            else:
                n = in_ap.shape[-1]
                h = (n * 5 // 8) & ~63 or n // 2
                nc.vector.tensor_copy(out=out_ap[:, 0:h], in_=in_ap[:, 0:h])
                nc.scalar.activation(out=out_ap[:, h:n], in_=in_ap[:, h:n],
                                     func=AF.Copy)

        BD_ENG = [0, 1, 0, 1]   # DVE, ACT, ...
        AC_ENG = [1, 0, 1, 0]

        with tc.tile_pool(name="score", bufs=4) as score, \
             tc.tile_pool(name="slabp", bufs=5) as slabp:
            owb = wo.tile([128, c.HPT * c.D], BF16, tag="owb")
            nc.gpsimd.dma_start(out=owb[:], in_=bass.AP(
                tensor=io["oww"].ap().tensor, offset=0,
                ap=[[c.D, 128], [128 * c.D, c.HPT], [1, c.D]]))

            pvs = [None, None]
            exp_q = []
            pv_q = []

            def emit_exp(st):
                sb, bdw, jmax = st["sb"], st["bdw"], st["jmax"]
                diag = bass.AP(tensor=bdw.tensor, offset=bdw.offset + 127,
                               ap=[[c.WB - 1, 128], [1, jmax]])
                nc.gpsimd.dma_start(out=sb[:, 0:jmax], in_=diag,
                                    accum_op=ALU.add)
                slab = slabp.tile([128, c.KL], BF16, tag="slab", name="slab")
                for hlo, hhi in [(0, jmax // 256 * 128), (jmax // 256 * 128, jmax)]:
                    nc.scalar.activation(out=sb[:, hlo:hhi], in_=sb[:, hlo:hhi],
                                         func=AF.Exp, scale=float(c.SCALE))
                    dstap = bass.AP(
                        tensor=slab.tensor, offset=slab.offset + hlo,
                        ap=[[c.KL, 128], [128, (hhi - hlo) // 128], [1, 128]])
                    nc.sync.dma_start(out=dstap, in_=sb[:, hlo:hhi],
                                      transpose=True)
                st["slab"] = slab

            def emit_pv(st):
                h, s, jmax, slab = st["h"], st["s"], st["jmax"], st["slab"]
                hp, hr = h // 2, (h % 2) * 64
                njt = jmax // 128
                if s % 4 == 0:
                    pv = pvs[h % 2] = psPV.tile([65, 512], F32, tag="pv",
                                                name="pspv")
                else:
                    pv = pvs[h % 2]
                col = (s % 4) * 128
                for jt in range(njt):
                    nc.tensor.matmul(
                        pv[0:65, col:col + 128],
                        vb[:, jt * c.VW + h * 65: jt * c.VW + h * 65 + 65],
                        slab[:, ts(jt, 128)],
                        start=(jt == 0), stop=(jt == njt - 1))
                if s % 4 == 3:
                    g = s // 4
                    rd = small.tile([1, 512], F32, tag="rd")
                    nc.vector.reciprocal(out=rd[0:1, :], in_=pv[64:65, :])
                    rdb = small.tile([64, 512], F32, tag="rdb")
                    src_b = bass.AP(tensor=rd.tensor, offset=rd.offset,
                                    ap=[[512, 1], [0, 64], [1, 512]])
                    nc.sync.dma_start(out=rdb[:], in_=src_b)
                    nc.vector.tensor_tensor(
                        out=attnT[hr:hr + 64, hp * c.Q + g * 512:
                                  hp * c.Q + (g + 1) * 512],
                        in0=pv[0:64, :], in1=rdb[:], op=ALU.mult)

            for hp in range(c.HPT):
                for s in range(8):
                  for h2 in range(2):
                    h = 2 * hp + h2
                    hr = h2 * 64
                    jmax, wst = c.jmax(s), c.wstart(s)
                    # --- BD window ---
                    bdw = score.tile([128, c.WB], BF16, tag="bdw")
                    for ci, (lo, hi) in enumerate(chunks(jmax)):
                        ps = psB.tile([128, 512], F32, tag="b", name="psbd")
                        nc.tensor.matmul(
                            ps[:, 0:hi - lo],
                            rrq[hr:hr + 64, hp * c.Q + s * 128: hp * c.Q + (s + 1) * 128],
                            rTp[hr:hr + 64, hp * c.KL + wst + lo: hp * c.KL + wst + hi],
                            start=True, stop=True)
                        drain(BD_ENG[h % 2], bdw[:, lo:hi], ps[:, 0:hi - lo])
                    nc.gpsimd.memset(bdw[:, jmax:jmax + 128], c.MASKV)
                    # --- AC ---
                    sb = score.tile([128, c.KL], BF16, tag="sb")
                    for lo, hi in chunks(jmax):
                        ps = psA.tile([128, 512], F32, tag="a", name="psac")
                        nc.tensor.matmul(
                            ps[:, 0:hi - lo],
                            rwq[hr:hr + 64, hp * c.Q + s * 128: hp * c.Q + (s + 1) * 128],
                            kT[hr:hr + 64, hp * c.KL + lo: hp * c.KL + hi],
                            start=True, stop=True)
                        drain(AC_ENG[h % 2], sb[:, lo:hi], ps[:, 0:hi - lo])
                    st = {"h": h, "s": s, "jmax": jmax, "sb": sb, "bdw": bdw}
                    exp_q.append(st)
                    pv_q.append(st)
                    if len(exp_q) > 1:
                        emit_exp(exp_q.pop(0))
                    if len(pv_q) > 2:
                        emit_pv(pv_q.pop(0))
            while exp_q:
                emit_exp(exp_q.pop(0))
            while pv_q:
                emit_pv(pv_q.pop(0))

        attk.release()

        # ============ phase D: o_proj (natural) -> ReduceScatter ============
        with tc.tile_pool(name="stD", bufs=3) as stD:
            for t in range(8):
                ob = stD.tile([128, c.D], BF16, tag="ob")
                for ci in range(2):
                    ps = psA.tile([128, 512], F32, tag="a", name="pso")
                    for k in range(c.HPT):
                        nc.tensor.matmul(
                            ps[:], attnT[:, k * c.Q + t * 128: k * c.Q + (t + 1) * 128],
                            owb[:, k * c.D + ci * 512: k * c.D + (ci + 1) * 512],
                            start=(k == 0), stop=(k == c.HPT - 1))
                    nc.scalar.activation(out=ob[:, ts(ci, 512)], in_=ps[:],
                                         func=AF.Copy)
                nc.sync.dma_start(out=io["rs_bin"][ts(t, 128), :], in_=ob[:])
            if collective:
                nc.gpsimd.collective_compute(
                    "ReduceScatter", ALU.add, replica_groups=rg,
                    ins=[io["rs_bin"].ap().opt()], outs=[io["rs_bout"].ap().opt()])
            else:
                nc.sync.dma_start(out=io["rs_bout"].ap().opt(),
                                  in_=io["rs_bin"].ap()[0:c.TOKF, :].opt())

            # w2 cast-load (during collective window)
            w2t = wff2.tile([128, 32 * c.D], BF16, tag="w2t")
            for g in range(4):
                nc.gpsimd.dma_start(
                    out=w2t[:, g * 8 * c.D:(g + 1) * 8 * c.D],
                    in_=bass.AP(tensor=io["ffw2"].ap().tensor,
                                offset=g * 8 * 128 * c.D,
                                ap=[[c.D, 128], [128 * c.D, 8], [1, c.D]]))
        atp.release()

        # ============ phase E: LN1 + FFN + LN2 ============
        phE = ctx.enter_context(tc.tile_pool(name="phE", bufs=1))
        eps_t = phE.tile([128, 1], F32, tag="eps")
        nc.vector.memset(eps_t[:], c.LN_EPS)
        lns = {}
        for nm in ("ln1g", "ln1b", "ln2g", "ln2b", "ffb2"):
            tl = phE.tile([128, c.D], BF16, tag=nm)
            bcast = bass.AP(tensor=io[nm].ap().tensor, offset=0,
                            ap=[[0, 128], [1, c.D]])
            nc.gpsimd.dma_start(out=tl[:], in_=bcast)
            lns[nm] = tl
        fb1 = phE.tile([128, c.DI // 128], F32, tag="fb1")
        nc.sync.dma_start(out=fb1[:], in_=bass.AP(
            tensor=io["ffb1"].ap().tensor, offset=0,
            ap=[[1, 128], [128, c.DI // 128]]))

        ffn = ctx.enter_context(tc.tile_pool(name="ffn", bufs=1))
        ntt = c.TOKF // 128
        ln1n = ffn.tile([128, ntt * c.D], F32, tag="ln1n")
        lnT = ffn.tile([128, c.DPT * c.TOKF], BF16, tag="lnT")
        hT = ffn.tile([128, 32 * c.TOKF], BF16, tag="hT")

        with tc.tile_pool(name="stE1", bufs=2) as stE1:
            for t in range(ntt):
                zt = stE1.tile([128, c.D], BF16, tag="zt")
                nc.sync.dma_start(out=zt[:], in_=io["rs_bout"][ts(t, 128), :])
                wv = stE1.tile([128, c.D], F32, tag="wv")
                nc.sync.dma_start(out=wv[:], in_=io["wres"][ts(t, 128), :])
                zf = stE1.tile([128, c.D], F32, tag="zf")
                nc.vector.tensor_tensor(out=zf[:], in0=wv[:], in1=zt[:],
                                        op=ALU.add)
                _layernorm_nat(nc, c, small, zf[:], eps_t,
                               lns["ln1g"], lns["ln1b"], ln1n[:, ts(t, c.D)])
                for g in range(2):
                    pst = psB.tile([128, 512], F32, tag="b", name="pstr")
                    for j in range(4):
                        nc.tensor.transpose(
                            pst[:, ts(j, 128)],
                            ln1n[:, t * c.D + (g * 4 + j) * 128:
                                 t * c.D + (g * 4 + j + 1) * 128],
                            identf[:])
                    dst = bass.AP(
                        tensor=lnT.tensor,
                        offset=lnT.offset + g * 4 * c.TOKF + t * 128,
                        ap=[[c.DPT * c.TOKF, 128], [c.TOKF, 4], [1, 128]])
                    nc.vector.tensor_copy(out=dst, in_=pst[:])

        def load_w2g(g):
            w2g = wff2.tile([128, 8 * c.D], BF16, tag="w2g", name="w2g")
            nc.gpsimd.dma_start(
                out=w2g[:],
                in_=bass.AP(tensor=io["ffw2"].ap().tensor,
                            offset=g * 8 * 128 * c.D,
                            ap=[[c.D, 128], [128 * c.D, 8], [1, c.D]]))
            return w2g

        w2cache = {}
        # FFN1: hT[di, tok] = relu(w1^T @ ln1^T + b1); w1 streamed in quarters
        for quarter in range(4):
            w1q = w1q_cur
            if quarter < 3:
                w1q_cur = load_w1q(quarter + 1)
            if quarter >= 2:
                g = quarter - 2
                w2cache[g] = load_w2g(g)
            for mm in range(8):
                m = quarter * 8 + mm
                pp, tg = (psA, "a") if mm % 2 == 0 else (psB, "b")
                ps = pp.tile([128, 512], F32, tag=tg, name="psf1")
                for k in range(c.DPT):
                    nc.tensor.matmul(
                        ps[:], w1q[:, k * 1024 + mm * 128: k * 1024 + (mm + 1) * 128],
                        lnT[:, ts(k, c.TOKF)],
                        start=(k == 0), stop=(k == c.DPT - 1))
                nc.scalar.activation(
                    out=hT[:, ts(m, c.TOKF)], in_=ps[:],
                    func=AF.Relu, bias=fb1[:, m:m + 1])

        # FFN2: natural out = hT^T @ w2 (+ residual + b2), LN2
        with tc.tile_pool(name="stE2", bufs=2) as stE2:
            for t in range(ntt):
                o2n = stE2.tile([128, c.D], F32, tag="o2n")
                for ci in range(2):
                    pp, tg = (psA, "a") if ci == 0 else (psC, "c")
                    ps = pp.tile([128, 512], F32, tag=tg, name="psf2")
                    for m in range(32):
                        nc.tensor.matmul(
                            ps[:], hT[:, m * c.TOKF + t * 128: m * c.TOKF + (t + 1) * 128],
                            w2t[:, m * c.D + ci * 512: m * c.D + (ci + 1) * 512],
                            start=(m == 0), stop=(m == 31))
                    nc.vector.tensor_tensor(
                        out=o2n[:, ts(ci, 512)], in0=ps[:],
                        in1=ln1n[:, t * c.D + ci * 512: t * c.D + (ci + 1) * 512],
                        op=ALU.add)
                nc.vector.tensor_tensor(out=o2n[:], in0=o2n[:],
                                        in1=lns["ffb2"][:], op=ALU.add)
                fin = stE2.tile([128, c.D], F32, tag="fin")
                _layernorm_nat(nc, c, small, o2n[:], eps_t,
                               lns["ln2g"], lns["ln2b"], fin[:])
                nc.sync.dma_start(out=io["out"][ts(t, 128), :], in_=fin[:])


def _layernorm_nat(nc, c, small, z, eps_t, g, b, out_dst):
    """LayerNorm over the free axis of z [128, D] fp32."""
    BN_FMAX = nc.vector.BN_STATS_FMAX
    d = z.shape[-1]
    sub = math.gcd(BN_FMAX, d)
    nsub = d // sub
    zr = z.rearrange("p (n f) -> p n f", f=sub)
    stats = small.tile([128, nsub, nc.vector.BN_STATS_DIM], F32, tag="bnst")
    for i in range(nsub):
        nc.vector.bn_stats(out=stats[:, i, :], in_=zr[:, i, :])
    mv = small.tile([128, nc.vector.BN_AGGR_DIM], F32, tag="bnag")
    nc.vector.bn_aggr(out=mv[:], in_=stats[:])
    mean, var = mv[:, 0:1], mv[:, 1:2]
    nc.scalar.activation(out=var, in_=var, func=AF.Sqrt, bias=eps_t[:], scale=1.0)
    nc.vector.reciprocal(out=var, in_=var)
    nc.vector.tensor_scalar(out=out_dst, in0=z, scalar1=mean, scalar2=var,
                            op0=ALU.subtract, op1=ALU.mult)
    nc.vector.tensor_tensor(out=out_dst, in0=out_dst, in1=g[:, 0:d], op=ALU.mult)
    nc.vector.tensor_tensor(out=out_dst, in0=out_dst, in1=b[:, 0:d], op=ALU.add)


# ============================================================
# host-side sharding + entry point
# ============================================================

def shard_inputs(inputs, c: Cfg = None):
    c = c or Cfg()
    w = np.asarray(inputs["w"], np.float32)
    r = np.asarray(inputs["r"], np.float32)
    mems = np.asarray(inputs["mems"], np.float32)
    qkv_w = np.asarray(inputs["qkv_w"], np.float32)
    r_net_w = np.asarray(inputs["r_net_w"], np.float32)
    o_w = np.asarray(inputs["o_w"], np.float32)
    r_w_bias = np.asarray(inputs["r_w_bias"], np.float32).reshape(-1)
    r_r_bias = np.asarray(inputs["r_r_bias"], np.float32).reshape(-1)
    NHD = qkv_w.shape[1] // 3
    in_maps = []
    for core in range(c.N_CORES):
        b, hh = core // 2, core % 2
        hsl = slice(hh * c.HD, (hh + 1) * c.HD)
        xw_c = np.concatenate([mems[:, b, :], w[:, b, :]], axis=0)
        qkvw_c = np.concatenate([qkv_w[:, j * NHD + hh * c.HD:
                                       j * NHD + (hh + 1) * c.HD]
                                 for j in range(3)], axis=1)
        in_maps.append({
            "xw": np.ascontiguousarray(xw_c),
            "r_in": np.ascontiguousarray(r[:, 0, :]),
            "qkvw": np.ascontiguousarray(qkvw_c),
            "rnetw": np.ascontiguousarray(r_net_w[:, hsl]),
            "oww": np.ascontiguousarray(o_w[hsl, :]),
            "rwb": np.ascontiguousarray(r_w_bias[hsl][None, :]),
            "rrb": np.ascontiguousarray(r_r_bias[hsl][None, :]),
            "ln1g": np.asarray(inputs["ln1_g"], np.float32).reshape(1, -1),
            "ln1b": np.asarray(inputs["ln1_b"], np.float32).reshape(1, -1),
            "ln2g": np.asarray(inputs["ln2_g"], np.float32).reshape(1, -1),
            "ln2b": np.asarray(inputs["ln2_b"], np.float32).reshape(1, -1),
            "ffw1": np.asarray(inputs["ff_w1"], np.float32),
            "ffb1": np.asarray(inputs["ff_b1"], np.float32).reshape(1, -1),
            "ffw2": np.asarray(inputs["ff_w2"], np.float32),
            "ffb2": np.asarray(inputs["ff_b2"], np.float32).reshape(1, -1),
            "wres": np.ascontiguousarray(w[hh * c.TOKF:(hh + 1) * c.TOKF, b, :]),
        })
    return in_maps


def unshard_output(results, inputs, c: Cfg = None):
    c = c or Cfg()
    w = np.asarray(inputs["w"])
    Q, B, D = w.shape
    out = np.zeros((Q, B, D), np.float32)
    for core in range(c.N_CORES):
        b, hh = core // 2, core % 2
        out[hh * c.TOKF:(hh + 1) * c.TOKF, b, :] = results[core]["out"]
    return out


_NC_CACHE = {}


def kernel(**inputs):
    if "nc" not in _NC_CACHE:
        _NC_CACHE["nc"] = build_kernel()
    nc = _NC_CACHE["nc"]
    in_maps = shard_inputs(inputs)
    from concourse.bass_utils import run_bass_kernel_spmd
    res = run_bass_kernel_spmd(nc, in_maps, core_ids=list(range(Cfg.N_CORES)))
    return unshard_output(res.results, inputs)
